# revision 29
# baseline (speedup 1.0000x reference)
"""Causal multi-head self-attention on 8 Trainium2 NeuronCores.

Sharding: core c = (b, g) with b = c // 4 (batch), g = c % 4 (head group).
Each core computes 4 of the 16 heads for one batch element:
  Q/K/V projections for feature rows 256g:256g+256 (Megatron column split),
  causal attention for those heads, and a partial output projection
  against Wo[:, 256g:256g+256] (row split). Host sums the 4 partials per batch.

All operands are pre-transposed on the host so the kernel never transposes:
  xt  = X[b].T          [D, S]   (d on partitions -> matmul contraction dim)
  wqt = Wq[rows].T      [D, 256]
  wkt = Wk[rows].T      [D, 256]
  wvt = Wv[rows].T      [D, 256]
  wot = Wo[:, cols].T   [256, D]

Attention is computed with scores transposed (S^T = K Q^T, kv on partitions)
so the PV matmul needs no transpose, and a ones-row appended to V yields the
softmax denominator inside the same accumulation.

Performance structure (engines are in-order queues, so emission order defines
overlap):
- Emission interleaves projection block sb with attention q-block sb so the
  scalar engine's exp stream (the 2nd bottleneck) starts ~15us in, not after
  all projections.
- Heads 2m / 2m+1 sit on partition halves 0-63 / 64-127, so their scores
  matmuls (contraction DH=64) issue back-to-back into the two PE row groups
  (tile_position (0,0)/(64,0)) and run concurrently, halving scores time.
- One exp instruction covers both heads of a pair; diagonal blocks only
  compute/exp/PV the causal q-suffix, and masking is one 128x128 triangle.
- O-projection of q-block qb is emitted spread through attention qb+1.
"""

import sys

sys.path.insert(0, "/opt/trn_rl_repo")

import numpy as np

B = 2
S = 2048
D = 1024
H = 16
DH = 64

NCORES = 8
GROUPS = 4            # head groups (cores per batch element)
HPC = H // GROUPS     # heads per core = 4
F = HPC * DH          # feature slice per core = 256

_nc_cache = {}


def _build(s=S):
    import concourse.bass as bass  # noqa: F401
    import concourse.mybir as mybir
    import concourse.tile as tile
    from concourse import bacc

    # Make Exp and Ln resolve to the single combined ACT table set so the
    # table-load pass emits one load instead of thrashing between the
    # exp-only and ln-only sets (1.28us per reload).
    if not getattr(bacc, "_act_tables_pinned", False):
        _orig_get_tables = bacc.get_activation_tables

        def _pinned_tables(arch):
            tables = _orig_get_tables(arch)
            exp = mybir.ActivationFunctionType.Exp
            ln = mybir.ActivationFunctionType.Ln
            for name, funcs in tables.items():
                if name != "natural_log_exp_and_others":
                    funcs.discard(exp)
                    funcs.discard(ln)
            return tables

        bacc.get_activation_tables = _pinned_tables
        bacc._act_tables_pinned = True

    f32 = mybir.dt.float32
    bf16 = mybir.dt.bfloat16
    dmm = bf16  # matmul operand dtype

    P = 128
    SB = 512               # q-block / free-dim block
    NSB = s // SB          # q blocks = 4
    KC = D // P            # 8 contraction chunks over D
    MC = F // P            # 2 head-pairs per core
    NSC = s // P           # s chunks of 128
    ND = D // SB           # 2 output column blocks

    nc = bacc.Bacc("TRN2", debug=False, num_devices=NCORES)
    xt = nc.dram_tensor("xt", [D, s], dmm, kind="ExternalInput").ap()
    wqt = nc.dram_tensor("wqt", [D, F], dmm, kind="ExternalInput").ap()
    wkt = nc.dram_tensor("wkt", [D, F], dmm, kind="ExternalInput").ap()
    wvt = nc.dram_tensor("wvt", [D, F], dmm, kind="ExternalInput").ap()
    wot = nc.dram_tensor("wot", [F, D], dmm, kind="ExternalInput").ap()
    y = nc.dram_tensor("y", [s, D], bf16, kind="ExternalOutput").ap()

    with tile.TileContext(nc) as tc:
        with (
            tc.tile_pool(name="w", bufs=1) as wpool,
            tc.tile_pool(name="const", bufs=1) as cpool,
            tc.tile_pool(name="xt", bufs=2) as xpool,
            tc.tile_pool(name="qkv", bufs=1) as qkvpool,
            tc.tile_pool(name="pt", bufs=4) as ptpool,
            tc.tile_pool(name="small", bufs=2) as spool,
            tc.tile_pool(name="yst", bufs=2) as ypool,
            tc.tile_pool(name="ps", bufs=1, space="PSUM") as pspool,
        ):
            # --- weights ---
            wq_s = wpool.tile([P, KC, F], dmm, name="wq_s")
            wk_s = wpool.tile([P, KC, F], dmm, name="wk_s")
            wv_s = wpool.tile([P, KC, F], dmm, name="wv_s")
            wo_s = wpool.tile([P, MC, D], dmm, name="wo_s")

            xt_r = xt.rearrange("(o p) s -> p o s", p=P)
            wq_r = wqt.rearrange("(o p) f -> p o f", p=P)
            wk_r = wkt.rearrange("(o p) f -> p o f", p=P)

            # DMA order: smallest first-needed pieces first so the first
            # matmuls start ~10us in and are paced by arriving chunks.
            xt_t0 = xpool.tile([P, KC, SB], dmm, name="xt_tile")
            nc.sync.dma_start(xt_t0[:, 0:2, :], xt_r[:, 0:2, 0:SB])
            nc.sync.dma_start(wq_s[:, :, 0:P], wq_r[:, :, 0:P])
            nc.sync.dma_start(wk_s[:, :, 0:P], wk_r[:, :, 0:P])
            nc.sync.dma_start(xt_t0[:, 2:4, :], xt_r[:, 2:4, 0:SB])
            nc.sync.dma_start(xt_t0[:, 4:6, :], xt_r[:, 4:6, 0:SB])
            nc.sync.dma_start(xt_t0[:, 6:8, :], xt_r[:, 6:8, 0:SB])
            nc.sync.dma_start(wq_s[:, :, P:F], wq_r[:, :, P:F])
            nc.sync.dma_start(wv_s[:], wvt.rearrange("(o p) f -> p o f", p=P))
            nc.sync.dma_start(wk_s[:, :, P:F], wk_r[:, :, P:F])
            nc.sync.dma_start(wo_s[:], wot.rearrange("(o p) f -> p o f", p=P))

            # --- PE warmup: dummy matmuls during the initial DMA wait so the
            # HAM clock-gate reaches 8/8 before real work arrives ---
            wu = cpool.tile([P, SB], dmm, name="wu")
            nc.gpsimd.memset(wu[:], 0.0)
            for i in range(4):
                psw = pspool.tile([P, SB], f32, name="psw", tag="p", bufs=1)
                for j in range(4):
                    nc.tensor.matmul(psw[:], wu[:, 0:P], wu[:], start=True, stop=True)

            # --- causal triangle mask for the in-diagonal 128x128 block ---
            # trimask[r, i, c] = 1.0 if r <= c else 0.0 (same for both heads i)
            trimask = cpool.tile([P, 2, P], dmm, name="trimask")
            for i in range(2):
                nc.gpsimd.memset(trimask[:, i, :], 1.0)
                nc.gpsimd.affine_select(
                    out=trimask[:, i, :],
                    in_=trimask[:, i, :],
                    compare_op=mybir.AluOpType.is_ge,
                    fill=0.0,
                    base=0,
                    pattern=[[1, P]],
                    channel_multiplier=-1,
                )

            # --- persistent activations ---
            qt_t = qkvpool.tile([P, MC, s], dmm, name="qt_t")   # Q^T
            kt_t = qkvpool.tile([P, MC, s], dmm, name="kt_t")   # K^T
            v_t = qkvpool.tile([P, NSC, HPC, DH + 1], dmm, name="v_t")  # V | 1
            ot_t = qkvpool.tile([P, MC, s], dmm, name="ot_t")   # attn out ^T
            ones_sb = cpool.tile([P, NSC * HPC], f32, name="ones_sb")
            nc.gpsimd.memset(ones_sb[:], 1.0)
            nc.vector.tensor_copy(
                out=v_t[:, :, :, DH:DH + 1],
                in_=ones_sb.rearrange("p (a b) -> p a b", b=HPC)[:, :, :, None],
            )

            scale = float(1.0 / np.sqrt(DH))
            LAG = 2

            xt_tiles = {0: xt_t0}

            def emit_xt_dma(sb):
                xt_tile = xpool.tile([P, KC, SB], dmm, name="xt_tile")
                nc.sync.dma_start(xt_tile[:], xt_r[:, :, sb * SB:(sb + 1) * SB])
                xt_tiles[sb] = xt_tile

            def emit_q_unit(sb, m):
                xt_tile = xt_tiles[sb]
                pp = pspool.tile([P, SB], f32, name="pp", tag="p", bufs=1)
                for k in range(KC):
                    nc.tensor.matmul(
                        pp[:],
                        (wq_s[:, k, m * P:(m + 1) * P]),
                        (xt_tile[:, k, :]),
                        start=(k == 0),
                        stop=(k == KC - 1),
                    )
                nc.vector.tensor_copy(
                    out=qt_t[:, m, sb * SB:(sb + 1) * SB], in_=pp[:]
                )

            def emit_k_unit(sb, m):
                xt_tile = xt_tiles[sb]
                pp = pspool.tile([P, SB], f32, name="pp", tag="p", bufs=1)
                for k in range(KC):
                    nc.tensor.matmul(
                        pp[:],
                        (wk_s[:, k, m * P:(m + 1) * P]),
                        (xt_tile[:, k, :]),
                        start=(k == 0),
                        stop=(k == KC - 1),
                    )
                nc.vector.tensor_copy(
                    out=kt_t[:, m, sb * SB:(sb + 1) * SB], in_=pp[:]
                )

            def emit_v_unit(sb, g):
                # one sc chunk of V
                xt_tile = xt_tiles[sb]
                sc = g
                pv = pspool.tile([P, SB], f32, name="pv", tag="p", bufs=1)
                for k in range(KC):
                    nc.tensor.matmul(
                        pv[:, 0:F],
                        (xt_tile[:, k, sc * P:(sc + 1) * P]),
                        (wv_s[:, k, :]),
                        start=(k == 0),
                        stop=(k == KC - 1),
                    )
                nc.vector.tensor_copy(
                    out=v_t[:, sb * 4 + g, :, 0:DH],
                    in_=pv[:, 0:F].rearrange("p (h d) -> p h d", d=DH),
                )

            ys_pend = {}

            def emit_oproj_unit(qb, sc, nb, last=False):
                # half an output-projection row chunk (one nb column block)
                if nb == 0:
                    ys_pend[sc] = ypool.tile([P, ND, SB], dmm, name="ys", bufs=2)
                ys = ys_pend[sc]
                if last:
                    # the attention po ring is free at the tail; borrow it so
                    # the final groups pipeline instead of serializing
                    py = pspool.tile([P, SB], f32, name="pyl", tag="o", bufs=3)
                else:
                    py = pspool.tile([P, SB], f32, name="py", tag="p", bufs=1)
                for o in range(MC):
                    nc.tensor.matmul(
                        py[:],
                        (ot_t[:, o, sc * P:(sc + 1) * P]),
                        (wo_s[:, o, nb * SB:(nb + 1) * SB]),
                        start=(o == 0),
                        stop=(o == MC - 1),
                    )
                if last and (sc + nb) % 2 == 0:
                    nc.scalar.copy(ys[:, nb, :], py[:])
                else:
                    nc.vector.tensor_copy(ys[:, nb, :], py[:])
                if nb == ND - 1:
                    nc.sync.dma_start(
                        y[sc * P:(sc + 1) * P, :], ys.rearrange("p a b -> p (a b)")
                    )

            def emit_attn_pair(qb, mo, weave=None):
                # weave: {chunk_index: [callable, ...]} filler work emitted
                # between this pair's chunks to keep the PE stream dense
                nkv = 4 * (qb + 1)
                weave = weave or {}
                hA, hB = 2 * mo, 2 * mo + 1
                po_A = pspool.tile([P, SB], f32, name="poA", tag="o", bufs=3)
                po_B = pspool.tile([P, SB], f32, name="poB", tag="o", bufs=3)
                pos = (po_A, po_B)
                pts = []

                def emit_pv(ci):
                    pt_c, q0_c = pts[ci]
                    for idx in range(2):
                        nc.tensor.matmul(
                            pos[idx][0:DH + 1, q0_c:],
                            (v_t[:, ci, 2 * mo + idx, :]),
                            (pt_c[:, idx, q0_c:]),
                            start=(ci == 0),
                            stop=(ci == nkv - 1),
                            skip_group_check=True,
                        )

                for c in range(nkv):
                    dj = c - 4 * qb
                    q0 = P * dj if dj >= 0 else 0
                    psS = pspool.tile([P, 2, SB], f32, name="psS", tag="sc", bufs=2)
                    for idx, r0 in ((0, 0), (1, DH)):
                        nc.tensor.matmul(
                            psS[:, idx, q0:],
                            (kt_t[r0:r0 + DH, mo, c * P:(c + 1) * P]),
                            (qt_t[r0:r0 + DH, mo, qb * SB + q0:(qb + 1) * SB]),
                            start=True,
                            stop=True,
                        )
                    pt = ptpool.tile([P, 2, SB], dmm, name="pt", bufs=6)
                    nc.scalar.activation(
                        pt[:, :, q0:],
                        psS[:, :, q0:],
                        mybir.ActivationFunctionType.Exp,
                        scale=scale,
                    )
                    if dj >= 0:
                        nc.vector.tensor_mul(
                            pt[:, :, q0:q0 + P], pt[:, :, q0:q0 + P], trimask[:]
                        )
                    pts.append((pt, q0))
                    if c >= LAG:
                        emit_pv(c - LAG)
                    for fn in weave.get(c, ()):
                        fn()
                for ci in range(max(0, nkv - LAG), nkv):
                    emit_pv(ci)
                for fn in weave.get(nkv, ()):
                    fn()

                # normalize via 1/d = exp(-ln d): ln/exp share an ACT table
                # set, keeping the slow divide pipes out entirely.
                for idx, h in ((0, hA), (1, hB)):
                    ld = spool.tile([1, SB], f32, name="ld", bufs=2)
                    nc.scalar.activation(
                        ld[:],
                        pos[idx][DH:DH + 1, :],
                        mybir.ActivationFunctionType.Ln,
                    )
                    rr = spool.tile([1, SB], f32, name="rr", bufs=2)
                    nc.scalar.activation(
                        rr[:], ld[:], mybir.ActivationFunctionType.Exp, scale=-1.0
                    )
                    rb = spool.tile([DH, SB], f32, name="rb", bufs=2)
                    nc.gpsimd.partition_broadcast(rb[:], rr[:])
                    prow = (h % MC) * DH
                    nc.vector.tensor_mul(
                        ot_t[prow:prow + DH, h // MC, qb * SB:(qb + 1) * SB],
                        pos[idx][0:DH, :],
                        rb[:],
                    )

            def Q(sb, m):
                return lambda: emit_q_unit(sb, m)

            def K(sb, m):
                return lambda: emit_k_unit(sb, m)

            def V(sb, g):
                return lambda: emit_v_unit(sb, g)

            def OP(qb, i, nb, last=False):
                return lambda: emit_oproj_unit(qb, 4 * qb + i, nb, last)

            def XD(sb):
                return lambda: emit_xt_dma(sb)

            def emit_dummy(n):
                # dependency-free matmuls: fill PE idle in ACT/DMA-bound
                # windows so the HAM clock-gate stays at full speed
                psw = pspool.tile([P, SB], f32, name="psw", tag="p", bufs=1)
                for j in range(n):
                    nc.tensor.matmul(psw[:], wu[:, 0:P], wu[:], start=True, stop=True)

            def DU(n=2):
                return lambda: emit_dummy(n)

            # Hand-scheduled emission: engines are in-order queues, so this
            # order IS the per-engine schedule. Scores start ~2 matmul groups
            # in; projection/output-projection units (~1 PSUM bank, ~1-2us of
            # PE work each) fill PE time between attention chunks so neither
            # the PE nor the scalar engine's exp stream starves. A unit woven
            # at chunk slot c must precede the PV of chunk c-2 it feeds.
            emit_q_unit(0, 0)
            emit_k_unit(0, 0)
            emit_attn_pair(0, 0, weave={
                0: [Q(0, 1), V(0, 0)], 1: [V(0, 1)], 2: [V(0, 2)],
                3: [V(0, 3), K(0, 1)], 4: [XD(1)],
            })
            emit_attn_pair(0, 1, weave={
                0: [Q(1, 0)], 1: [K(1, 0), DU(2)], 2: [Q(1, 1), DU(2)],
                3: [K(1, 1)], 4: [XD(2)],
            })
            emit_attn_pair(1, 0, weave={
                0: [V(1, 0)], 1: [V(1, 1)], 2: [V(1, 2)], 3: [V(1, 3)],
                4: [Q(2, 0)], 5: [K(2, 0)], 6: [OP(0, 0, 0)], 7: [OP(0, 0, 1)],
                8: [OP(0, 1, 0)],
            })
            emit_attn_pair(1, 1, weave={
                0: [Q(2, 1)], 1: [K(2, 1)], 2: [OP(0, 1, 1)], 3: [OP(0, 2, 0)],
                4: [OP(0, 2, 1)], 5: [OP(0, 3, 0)], 6: [OP(0, 3, 1)],
                7: [XD(3)],
            })
            emit_attn_pair(2, 0, weave={
                0: [V(2, 0)], 1: [V(2, 1)], 2: [V(2, 2)], 3: [V(2, 3)],
                5: [Q(3, 0)], 7: [K(3, 0)], 9: [OP(1, 0, 0)], 10: [OP(1, 0, 1)],
                11: [OP(1, 1, 0)], 12: [OP(1, 1, 1)],
            })
            emit_attn_pair(2, 1, weave={
                0: [Q(3, 1)], 2: [K(3, 1)], 4: [OP(1, 2, 0)], 6: [OP(1, 2, 1)],
                8: [OP(1, 3, 0)], 10: [OP(1, 3, 1)],
            })
            emit_attn_pair(3, 0, weave={
                0: [V(3, 0)], 1: [V(3, 1)], 2: [V(3, 2)], 3: [V(3, 3)],
                5: [OP(2, 0, 0)], 7: [OP(2, 0, 1)], 9: [OP(2, 1, 0)],
                10: [OP(2, 1, 1)], 11: [OP(2, 2, 0)], 12: [OP(2, 2, 1)],
                13: [OP(2, 3, 0)], 14: [OP(2, 3, 1)],
            })
            emit_attn_pair(3, 1, weave={
                1: [DU(2)], 3: [DU(2)], 5: [DU(2)], 7: [DU(2)], 9: [DU(2)],
                11: [DU(2)], 13: [DU(2)], 15: [DU(2)], 16: [DU(4)],
            })
            emit_dummy(8)
            for i in range(4):
                for nb in range(ND):
                    emit_oproj_unit(3, 12 + i, nb, last=True)

    nc.compile()
    return nc


def _get_nc(s=S):
    if s not in _nc_cache:
        _nc_cache[s] = _build(s)
    return _nc_cache[s]


def make_in_maps(in_features, Wq, Wk, Wv, Wo):
    """Shard full inputs into 8 per-core input dicts (bf16 operands)."""
    import ml_dtypes
    bf = ml_dtypes.bfloat16
    x = np.asarray(in_features, dtype=np.float32)
    wq = np.asarray(Wq, dtype=np.float32)
    wk = np.asarray(Wk, dtype=np.float32)
    wv = np.asarray(Wv, dtype=np.float32)
    wo = np.asarray(Wo, dtype=np.float32)

    xts = [np.ascontiguousarray(x[b].T) for b in range(B)]
    in_maps = []
    for c in range(NCORES):
        b, g = divmod(c, GROUPS)
        rows = slice(g * F, (g + 1) * F)
        in_maps.append(
            {
                "xt": xts[b].astype(bf),
                "wqt": np.ascontiguousarray(wq[rows, :].T).astype(bf),
                "wkt": np.ascontiguousarray(wk[rows, :].T).astype(bf),
                "wvt": np.ascontiguousarray(wv[rows, :].T).astype(bf),
                "wot": np.ascontiguousarray(wo[:, rows].T).astype(bf),
            }
        )
    return in_maps


def combine_outputs(results):
    """Sum the 4 partial Y per batch element back into [B, S, D]."""
    out = np.zeros((B, S, D), dtype=np.float32)
    for c in range(NCORES):
        b = c // GROUPS
        out[b] += np.asarray(results[c]["y"]).astype(np.float32)
    return out


def kernel(in_features, Wq, Wk, Wv, Wo):
    from concourse import bass_utils

    nc = _get_nc()
    in_maps = make_in_maps(in_features, Wq, Wk, Wv, Wo)
    res = bass_utils.run_bass_kernel_spmd(nc, in_maps, core_ids=list(range(NCORES)))
    return combine_outputs(res.results)
